# Initial kernel scaffold
#
"""Trainium2 Bass kernel for DiagonalVectorSpinGlassAttention.

Math (derived analytically from the reference; verified vs jax.jacrev to
rel err 6e-7): with xs = per-head unit-normalized x, for each head h

    q = xs_flat @ Wq_h^T          k = xs_flat @ Wk_h^T      (n, 64)
    P = softmax(q k^T, rows)
    out[:, h*64:(h+1)*64] = (P @ k) @ Wq_hh + (P^T @ q) @ Wk_hh + c0 * xs_h

where Wq_hh / Wk_hh are the (64, 64) diagonal blocks of W_qk that map head-h
input columns, and c0 = 0.5 / v with v = (0.5 + sqrt(1.25)) / 2 (the
discriminant of the reference's quadratic collapses to 0.25 + beta^2 * |x|^2
and |x|^2 == 1 after normalization, making the local term a constant scale).
The mask is all-True in this problem, so it is a no-op.

Sharding: head-parallel over 8 cores, 2 head-slots per core (cores 0-3 get 2
real heads, cores 4-7 get 1 real head + 1 dummy slot).
"""

import numpy as np

import concourse.bass as bass
import concourse.tile as tile
from concourse import mybir
from concourse import bass_utils
from concourse.masks import make_identity

H, D = 12, 64
N = 1024
DIM = H * D  # 768
P = 128
NT = N // P  # 8 token tiles
NC = DIM // P  # 6 contraction tiles
NCORES = 8
SLOTS = 2
C0 = np.float32(0.5 / ((0.5 + np.sqrt(1.25)) / 2.0))  # 0.618034
F32 = mybir.dt.float32

# head assignment: slot 0 = heads 0..7, slot 1 = heads 8..11 on cores 0..3
HEAD_MAP = [[c, c + 8 if c < 4 else -1] for c in range(NCORES)]

_cache = {}


def _ts(i, size):
    return slice(i * size, (i + 1) * size)


def _build_kernel_body(tc):
    nc = tc.nc
    Exp = mybir.ActivationFunctionType.Exp
    mult = mybir.AluOpType.mult
    add = mybir.AluOpType.add

    at_d = nc.dram_tensor("at", (DIM, N), F32, kind="ExternalInput").ap()
    wqk_d = nc.dram_tensor("wqk", (SLOTS, DIM, 128), F32, kind="ExternalInput").ap()
    whh_d = nc.dram_tensor("whh", (SLOTS, 64, 128), F32, kind="ExternalInput").ap()
    ats_d = nc.dram_tensor("ats", (SLOTS, 64, N), F32, kind="ExternalInput").ap()
    c0i_d = nc.dram_tensor("c0i", (64, 64), F32, kind="ExternalInput").ap()
    out_d = nc.dram_tensor("out", (SLOTS, N, 64), F32, kind="ExternalOutput").ap()

    import contextlib

    ctx = contextlib.ExitStack()
    with ctx:
        const = ctx.enter_context(tc.tile_pool(name="const", bufs=1))
        wpool = ctx.enter_context(tc.tile_pool(name="wpool", bufs=2))
        spool = ctx.enter_context(tc.tile_pool(name="spool", bufs=1))
        small = ctx.enter_context(tc.tile_pool(name="small", bufs=2))
        pp_s = ctx.enter_context(tc.tile_pool(name="pp_s", bufs=3, space="PSUM"))
        pp_uw = ctx.enter_context(tc.tile_pool(name="pp_uw", bufs=2, space="PSUM"))
        pp_sm = ctx.enter_context(tc.tile_pool(name="pp_sm", bufs=2, space="PSUM"))

        # constants: 128x128 identity (for PE transpose), c0*I_64
        ident = const.tile([P, P], F32)
        make_identity(nc, ident[:])
        c0i_sb = const.tile([64, 64], F32)
        nc.sync.dma_start(c0i_sb[:], c0i_d)

        # A^T: (768, 1024) -> (128, 6, 1024)
        at_sb = const.tile([P, NC, N], F32)
        at3 = at_d.rearrange("(c p) m -> p c m", p=P)
        for c in range(NC):
            nc.sync.dma_start(at_sb[:, c, :], at3[:, c, :])

        for s in range(SLOTS):
            # ---- per-slot weights ----
            wqk_sb = wpool.tile([P, NC, 128], F32, tag="wqk")
            nc.sync.dma_start(wqk_sb[:], wqk_d[s].rearrange("(c p) m -> p c m", p=P))
            whh_sb = wpool.tile([64, 128], F32, tag="whh")
            nc.sync.dma_start(whh_sb[:], whh_d[s])
            atsT_sb = wpool.tile([64, N], F32, tag="ats")
            nc.sync.dma_start(atsT_sb[:], ats_d[s])

            # ---- projection: qkT = [q^T; k^T] (128, 1024) ----
            ps_qk = pp_s.tile([P, N], F32, tag="ps_qk")
            for hf in range(2):
                for c in range(NC):
                    nc.tensor.matmul(
                        ps_qk[:, _ts(hf, 512)],
                        lhsT=wqk_sb[:, c, :],
                        rhs=at_sb[:, c, _ts(hf, 512)],
                        start=(c == 0),
                        stop=(c == NC - 1),
                    )
            qkT = spool.tile([P, N], F32, tag="qkT")
            nc.vector.tensor_copy(qkT[:], ps_qk[:])
            # swapped copy [k^T; q^T] so both q^T and k^T exist at partitions 0-63
            kqT = spool.tile([P, N], F32, tag="kqT")
            nc.sync.dma_start(kqT[0:64, :], qkT[64:128, :])
            nc.sync.dma_start(kqT[64:128, :], qkT[0:64, :])

            # ---- token-layout q|k via PE transpose: qk_tok (128, 8, 128) ----
            qk_tok = spool.tile([P, NT, P], F32, tag="qk_tok")
            for t in range(NT):
                ps_tp = pp_sm.tile([P, P], F32, tag="ps_sm")
                nc.tensor.transpose(ps_tp[:], qkT[:, _ts(t, P)], ident[:])
                nc.vector.tensor_copy(qk_tok[:, t, :], ps_tp[:])

            # ---- E2 = exp(k q^T) (j on partitions) ----
            e2 = spool.tile([P, NT, N], F32, tag="e2")
            for t in range(NT):
                ps_s2 = pp_s.tile([P, 512], F32, tag="ps_s")
                ps_s2b = pp_s.tile([P, 512], F32, tag="ps_s")
                nc.tensor.matmul(ps_s2[:], lhsT=kqT[0:64, _ts(t, P)],
                                 rhs=qkT[0:64, 0:512], start=True, stop=True)
                nc.tensor.matmul(ps_s2b[:], lhsT=kqT[0:64, _ts(t, P)],
                                 rhs=qkT[0:64, 512:1024], start=True, stop=True)
                nc.scalar.activation(e2[:, t, 0:512], ps_s2[:], Exp)
                nc.scalar.activation(e2[:, t, 512:1024], ps_s2b[:], Exp)

            # ---- E1 = exp(q k^T) (i on partitions), rowsum r via accum ----
            e1 = spool.tile([P, NT, N], F32, tag="e1")
            racc = small.tile([P, NT, 2], F32, tag="racc")
            for t in range(NT):
                ps_s1 = pp_s.tile([P, 512], F32, tag="ps_s")
                ps_s1b = pp_s.tile([P, 512], F32, tag="ps_s")
                nc.tensor.matmul(ps_s1[:], lhsT=qkT[0:64, _ts(t, P)],
                                 rhs=kqT[0:64, 0:512], start=True, stop=True)
                nc.tensor.matmul(ps_s1b[:], lhsT=qkT[0:64, _ts(t, P)],
                                 rhs=kqT[0:64, 512:1024], start=True, stop=True)
                nc.scalar.activation(e1[:, t, 0:512], ps_s1[:], Exp,
                                     accum_out=racc[:, t, 0:1])
                nc.scalar.activation(e1[:, t, 512:1024], ps_s1b[:], Exp,
                                     accum_out=racc[:, t, 1:2])

            # r = half sums added; recip = 1/r  (token-partition layout (128, 8))
            r_tok = small.tile([P, NT], F32, tag="r_tok")
            nc.vector.tensor_add(r_tok[:], racc[:, :, 0], racc[:, :, 1])
            recip = small.tile([P, NT], F32, tag="recip")
            nc.vector.reciprocal(recip[:], r_tok[:])

            # q' = q / r (token layout)
            qp = spool.tile([P, NT, 64], F32, tag="qp")
            for t in range(NT):
                nc.vector.tensor_scalar_mul(qp[:, t, :], qk_tok[:, t, 0:64],
                                            recip[:, t : t + 1])

            # ---- u_raw^T = k^T E2 (accumulate over j tiles) -> (64, 1024) ----
            uT = spool.tile([64, N], F32, tag="uT")
            for hf in range(2):
                ps_u = pp_uw.tile([64, 512], F32, tag="ps_uw")
                for t in range(NT):
                    nc.tensor.matmul(ps_u[:], lhsT=qk_tok[:, t, 64:128],
                                     rhs=e2[:, t, _ts(hf, 512)],
                                     start=(t == 0), stop=(t == NT - 1))
                nc.vector.tensor_copy(uT[:, _ts(hf, 512)], ps_u[:])

            # ---- w^T = q'^T E1 (accumulate over i tiles) -> (64, 1024) ----
            wT = spool.tile([64, N], F32, tag="wT")
            for hf in range(2):
                ps_w = pp_uw.tile([64, 512], F32, tag="ps_uw")
                for t in range(NT):
                    nc.tensor.matmul(ps_w[:], lhsT=qp[:, t, :],
                                     rhs=e1[:, t, _ts(hf, 512)],
                                     start=(t == 0), stop=(t == NT - 1))
                nc.vector.tensor_copy(wT[:, _ts(hf, 512)], ps_w[:])

            # ---- final: out_t = (uT_t^T @ Wq_hh) * recip + wT_t^T @ Wk_hh + c0*xs
            for t in range(NT):
                ps_f = pp_sm.tile([P, P], F32, tag="ps_sm")
                # u-term (cols 0:64), unscaled
                nc.tensor.matmul(ps_f[:, 0:64], lhsT=uT[:, _ts(t, P)],
                                 rhs=whh_sb[:, 0:64], start=True, stop=True)
                # rest (cols 64:128): w-term + c0*xs
                nc.tensor.matmul(ps_f[:, 64:128], lhsT=wT[:, _ts(t, P)],
                                 rhs=whh_sb[:, 64:128], start=True, stop=False)
                nc.tensor.matmul(ps_f[:, 64:128], lhsT=atsT_sb[:, _ts(t, P)],
                                 rhs=c0i_sb[:], start=False, stop=True)
                rest = small.tile([P, 64], F32, tag="rest")
                nc.scalar.copy(rest[:], ps_f[:, 64:128])
                out_t = small.tile([P, 64], F32, tag="out_t")
                nc.vector.scalar_tensor_tensor(out_t[:], ps_f[:, 0:64],
                                               recip[:, t : t + 1], rest[:],
                                               mult, add)
                nc.sync.dma_start(out_d[s, _ts(t, P), :], out_t[:])


def _get_nc():
    if "nc" not in _cache:
        nc = bass.Bass("TRN2", debug=False, target_bir_lowering=False,
                       num_devices=NCORES)
        with tile.TileContext(nc) as tc:
            _build_kernel_body(tc)
        _cache["nc"] = nc
    return _cache["nc"]


def _prep_inputs(x, W_qk):
    x = np.asarray(x, dtype=np.float32)
    W = np.asarray(W_qk, dtype=np.float32)
    n = x.shape[0]
    xh = x.reshape(n, H, D)
    nrm = np.sqrt(np.sum(xh * xh, axis=-1, keepdims=True, dtype=np.float32))
    xh = (xh / nrm).astype(np.float32)
    A = np.ascontiguousarray(xh.reshape(n, DIM))
    AT = np.ascontiguousarray(A.T)  # (768, 1024)

    c0i = (C0 * np.eye(64, dtype=np.float32)).astype(np.float32)

    in_maps = []
    for c in range(NCORES):
        wqk = np.zeros((SLOTS, DIM, 128), dtype=np.float32)
        whh = np.zeros((SLOTS, 64, 128), dtype=np.float32)
        ats = np.zeros((SLOTS, 64, N), dtype=np.float32)
        for s in range(SLOTS):
            h = HEAD_MAP[c][s]
            if h < 0:
                h = 0  # dummy slot computes head 0; output ignored
            Wq_h = W[h * D : (h + 1) * D, :]          # (64, 768)
            Wk_h = W[DIM + h * D : DIM + (h + 1) * D, :]
            wqk[s, :, 0:64] = Wq_h.T
            wqk[s, :, 64:128] = Wk_h.T
            whh[s, :, 0:64] = Wq_h[:, h * D : (h + 1) * D]
            whh[s, :, 64:128] = Wk_h[:, h * D : (h + 1) * D]
            ats[s] = AT[h * D : (h + 1) * D, :]
        in_maps.append({
            "at": AT,
            "wqk": np.ascontiguousarray(wqk),
            "whh": np.ascontiguousarray(whh),
            "ats": np.ascontiguousarray(ats),
            "c0i": c0i,
        })
    return in_maps


def kernel(x, mask, W_qk, trace=False):
    nc = _get_nc()
    in_maps = _prep_inputs(x, W_qk)
    res = bass_utils.run_bass_kernel_spmd(
        nc, in_maps, core_ids=list(range(NCORES)), trace=trace
    )
    _cache["last_results"] = res

    out = np.empty((N, DIM), dtype=np.float32)
    for c in range(NCORES):
        for s in range(SLOTS):
            h = HEAD_MAP[c][s]
            if h >= 0:
                out[:, h * D : (h + 1) * D] = res.results[c]["out"][s]
    return out


# revision 17
# speedup vs baseline: 1.3040x; 1.3040x over previous
"""Trainium2 Bass kernel for DiagonalVectorSpinGlassAttention.

Math (derived analytically from the reference; verified vs jax.jacrev to
rel err 6e-7): with xs = per-head unit-normalized x, for each head h

    q = xs_flat @ Wq_h^T          k = xs_flat @ Wk_h^T      (n, 64)
    P = softmax(q k^T, rows)
    out[:, h*64:(h+1)*64] = (P @ k) @ Wq_hh + (P^T @ q) @ Wk_hh + c0 * xs_h

where Wq_hh / Wk_hh are the (64, 64) diagonal blocks of W_qk that map head-h
input columns, and c0 = 0.5 / v with v = (0.5 + sqrt(1.25)) / 2 (the
discriminant of the reference's quadratic collapses to 0.25 + beta^2 * |x|^2
and |x|^2 == 1 after normalization, making the local term a constant scale).
The mask is all-True in this problem, so it is a no-op.

Sharding: head-parallel over 8 cores, 2 head-slots per core (cores 0-3 get 2
real heads, cores 4-7 get 1 real head + 1 dummy slot).
"""

import numpy as np

import concourse.bass as bass
import concourse.tile as tile
from concourse import mybir
from concourse import bass_utils
from concourse.masks import make_identity

H, D = 12, 64
N = 1024
DIM = H * D  # 768
P = 128
NT = N // P  # 8 token tiles
NC = DIM // P  # 6 contraction tiles
NCORES = 8
SLOTS = 2
C0 = np.float32(0.5 / ((0.5 + np.sqrt(1.25)) / 2.0))  # 0.618034
F32 = mybir.dt.float32

# head assignment: slot 0 = heads 0..7, slot 1 = heads 8..11 on cores 0..3
HEAD_MAP = [[c, c + 8 if c < 4 else -1] for c in range(NCORES)]

_cache = {}


def _ts(i, size):
    return slice(i * size, (i + 1) * size)


def _build_kernel_body(tc):
    import os
    STAGE = int(os.environ.get("K_STAGE", "9"))
    REPS = int(os.environ.get("K_REPS", "1"))
    nc = tc.nc
    Exp = mybir.ActivationFunctionType.Exp
    mult = mybir.AluOpType.mult
    add = mybir.AluOpType.add

    at_d = nc.dram_tensor("at", (DIM, N), F32, kind="ExternalInput").ap()
    wqk_d = nc.dram_tensor("wqk", (SLOTS, DIM, 128), F32, kind="ExternalInput").ap()
    whh_d = nc.dram_tensor("whh", (SLOTS, 64, 128), F32, kind="ExternalInput").ap()
    ats_d = nc.dram_tensor("ats", (SLOTS, 64, N), F32, kind="ExternalInput").ap()
    c0i_d = nc.dram_tensor("c0i", (64, 64), F32, kind="ExternalInput").ap()
    out_d = nc.dram_tensor("out", (SLOTS, N, 64), F32, kind="ExternalOutput").ap()

    import contextlib

    ctx = contextlib.ExitStack()
    with ctx:
        const = ctx.enter_context(tc.tile_pool(name="const", bufs=1))
        wpool = ctx.enter_context(tc.tile_pool(name="wpool", bufs=2))
        spool = ctx.enter_context(tc.tile_pool(name="spool", bufs=1))
        small = ctx.enter_context(tc.tile_pool(name="small", bufs=2))
        pp_s = ctx.enter_context(tc.tile_pool(name="pp_s", bufs=2, space="PSUM"))
        pp_uw = ctx.enter_context(tc.tile_pool(name="pp_uw", bufs=2, space="PSUM"))
        pp_sm = ctx.enter_context(tc.tile_pool(name="pp_sm", bufs=2, space="PSUM"))

        # constants: 128x128 identity (for PE transpose), c0*I_64
        ident = const.tile([P, P], F32)
        make_identity(nc, ident[:])
        c0i_sb = const.tile([64, 64], F32)
        nc.sync.dma_start(c0i_sb[:], c0i_d)

        # A^T: (768, 1024) -> (128, 6, 1024)
        at_sb = const.tile([P, NC, N], F32)
        at3 = at_d.rearrange("(c p) m -> p c m", p=P)
        for c in range(NC):
            nc.sync.dma_start(at_sb[:, c, :], at3[:, c, :])

        for s in [s_ for _ in range(REPS) for s_ in range(SLOTS)]:
            # ---- per-slot weights ----
            wqk_sb = wpool.tile([P, NC, 128], F32, tag="wqk")
            nc.sync.dma_start(wqk_sb[:], wqk_d[s].rearrange("(c p) m -> p c m", p=P))
            whh_sb = wpool.tile([64, 128], F32, tag="whh")
            nc.sync.dma_start(whh_sb[:], whh_d[s])
            atsT_sb = wpool.tile([64, N], F32, tag="ats")
            nc.sync.dma_start(atsT_sb[:], ats_d[s])

            # ---- projection: qkT = [q^T; k^T] (128, 1024) ----
            qkT = spool.tile([P, N], F32, tag="qkT")
            for hf in range(2):
                ps_qk = pp_s.tile([P, 512], F32, tag="ps_s")
                for c in range(NC):
                    nc.tensor.matmul(
                        ps_qk[:],
                        lhsT=wqk_sb[:, c, :],
                        rhs=at_sb[:, c, _ts(hf, 512)],
                        start=(c == 0),
                        stop=(c == NC - 1),
                    )
                nc.vector.tensor_copy(qkT[:, _ts(hf, 512)], ps_qk[:])
            # swapped copy [k^T; q^T] so both q^T and k^T exist at partitions 0-63
            kqT = spool.tile([P, N], F32, tag="kqT")
            nc.sync.dma_start(kqT[0:64, :], qkT[64:128, :])
            nc.sync.dma_start(kqT[64:128, :], qkT[0:64, :])

            if STAGE == 1:
                dbg = small.tile([P, 64], F32, tag="out_t")
                nc.vector.tensor_copy(dbg[:], kqT[:, 0:64])
                nc.sync.dma_start(out_d[s, 0:P, :], dbg[:])
                continue

            # ---- token-layout q|k via PE transpose: qk_tok (128, 8, 128) ----
            qk_tok = spool.tile([P, NT, P], F32, tag="qk_tok")
            for t in range(NT):
                ps_tp = pp_s.tile([P, P], F32, tag="ps_s")
                nc.tensor.transpose(ps_tp[:], qkT[:, _ts(t, P)], ident[:])
                nc.vector.tensor_copy(qk_tok[:, t, :], ps_tp[:])

            if STAGE == 2:
                dbg = small.tile([P, 64], F32, tag="out_t")
                nc.vector.tensor_copy(dbg[:], qk_tok[:, 0, 0:64])
                nc.sync.dma_start(out_d[s, 0:P, :], dbg[:])
                continue

            # ---- E2 = exp(k q^T) (j on partitions) ----
            e2 = spool.tile([P, NT, N], F32, tag="e2")
            for t in range(NT):
                ps_s2 = pp_s.tile([P, 512], F32, tag="ps_s")
                ps_s2b = pp_s.tile([P, 512], F32, tag="ps_s")
                nc.tensor.matmul(ps_s2[:], lhsT=kqT[0:64, _ts(t, P)],
                                 rhs=qkT[0:64, 0:512], start=True, stop=True)
                nc.tensor.matmul(ps_s2b[:], lhsT=kqT[0:64, _ts(t, P)],
                                 rhs=qkT[0:64, 512:1024], start=True, stop=True)
                nc.scalar.activation(e2[:, t, 0:512], ps_s2[:], Exp)
                nc.scalar.activation(e2[:, t, 512:1024], ps_s2b[:], Exp)

            if STAGE == 3:
                dbg = small.tile([P, 64], F32, tag="out_t")
                nc.vector.tensor_copy(dbg[:], e2[:, 0, 0:64])
                nc.sync.dma_start(out_d[s, 0:P, :], dbg[:])
                continue

            # ---- E1 = exp(q k^T) (i on partitions), rowsum r via accum ----
            e1 = spool.tile([P, NT, N], F32, tag="e1")
            racc = small.tile([P, NT, 2], F32, tag="racc")
            for t in range(NT):
                ps_s1 = pp_s.tile([P, 512], F32, tag="ps_s")
                ps_s1b = pp_s.tile([P, 512], F32, tag="ps_s")
                nc.tensor.matmul(ps_s1[:], lhsT=qkT[0:64, _ts(t, P)],
                                 rhs=kqT[0:64, 0:512], start=True, stop=True)
                nc.tensor.matmul(ps_s1b[:], lhsT=qkT[0:64, _ts(t, P)],
                                 rhs=kqT[0:64, 512:1024], start=True, stop=True)
                nc.scalar.activation(e1[:, t, 0:512], ps_s1[:], Exp,
                                     accum_out=racc[:, t, 0:1])
                nc.scalar.activation(e1[:, t, 512:1024], ps_s1b[:], Exp,
                                     accum_out=racc[:, t, 1:2])

            # r = half sums added; recip = 1/r  (token-partition layout (128, 8))
            r_tok = small.tile([P, NT], F32, tag="r_tok")
            nc.vector.tensor_add(r_tok[:], racc[:, :, 0], racc[:, :, 1])
            recip = small.tile([P, NT], F32, tag="recip")
            nc.vector.reciprocal(recip[:], r_tok[:])

            # q' = q / r (token layout)
            qp = spool.tile([P, NT, 64], F32, tag="qp")
            for t in range(NT):
                nc.vector.tensor_scalar_mul(qp[:, t, :], qk_tok[:, t, 0:64],
                                            recip[:, t : t + 1])

            if STAGE == 4:
                dbg = small.tile([P, 64], F32, tag="out_t")
                nc.vector.tensor_copy(dbg[:], qp[:, 0, :])
                nc.sync.dma_start(out_d[s, 0:P, :], dbg[:])
                continue

            # ---- u_raw^T = k^T E2 (accumulate over j tiles) -> (64, 1024) ----
            uT = spool.tile([64, N], F32, tag="uT")
            for hf in range(2):
                ps_u = pp_uw.tile([64, 512], F32, tag="ps_uw")
                for t in range(NT):
                    nc.tensor.matmul(ps_u[:], lhsT=qk_tok[:, t, 64:128],
                                     rhs=e2[:, t, _ts(hf, 512)],
                                     start=(t == 0), stop=(t == NT - 1))
                nc.vector.tensor_copy(uT[:, _ts(hf, 512)], ps_u[:])

            # ---- w^T = q'^T E1 (accumulate over i tiles) -> (64, 1024) ----
            wT = spool.tile([64, N], F32, tag="wT")
            for hf in range(2):
                ps_w = pp_uw.tile([64, 512], F32, tag="ps_uw")
                for t in range(NT):
                    nc.tensor.matmul(ps_w[:], lhsT=qp[:, t, :],
                                     rhs=e1[:, t, _ts(hf, 512)],
                                     start=(t == 0), stop=(t == NT - 1))
                nc.vector.tensor_copy(wT[:, _ts(hf, 512)], ps_w[:])

            if STAGE == 5:
                dbg = small.tile([P, 64], F32, tag="out_t")
                nc.vector.scalar_tensor_tensor(dbg[0:64, :], uT[:, 0:64], 1.0,
                                               wT[:, 0:64], mult, add)
                nc.sync.dma_start(out_d[s, 0:64, :], dbg[0:64, :])
                continue

            # ---- final: out_t = (uT_t^T @ Wq_hh) * recip + wT_t^T @ Wk_hh + c0*xs
            for t in range(NT):
                # u-term, unscaled (own PSUM bank)
                ps_fu = pp_sm.tile([P, 64], F32, tag="ps_fu")
                nc.tensor.matmul(ps_fu[:], lhsT=uT[:, _ts(t, P)],
                                 rhs=whh_sb[:, 0:64], start=True, stop=True)
                # rest: w-term + c0*xs (own PSUM bank)
                ps_fr = pp_sm.tile([P, 64], F32, tag="ps_fr")
                nc.tensor.matmul(ps_fr[:], lhsT=wT[:, _ts(t, P)],
                                 rhs=whh_sb[:, 64:128], start=True, stop=False)
                nc.tensor.matmul(ps_fr[:], lhsT=atsT_sb[:, _ts(t, P)],
                                 rhs=c0i_sb[:], start=False, stop=True)
                out_t = small.tile([P, 64], F32, tag="out_t")
                if STAGE == 6:
                    nc.vector.tensor_copy(out_t[:], ps_fu[:])
                elif STAGE == 7:
                    nc.vector.tensor_scalar_mul(out_t[:], ps_fu[:],
                                                recip[:, t : t + 1])
                else:
                    rest = small.tile([P, 64], F32, tag="rest")
                    nc.scalar.copy(rest[:], ps_fr[:])
                    nc.vector.tensor_scalar_mul(out_t[:], ps_fu[:],
                                                recip[:, t : t + 1])
                    nc.vector.tensor_add(out_t[:], out_t[:], rest[:])
                nc.sync.dma_start(out_d[s, _ts(t, P), :], out_t[:])


def _split_multi_waits(nc, limit=1):
    """The walrus build in this container encodes at most one sync-wait per
    instruction. Move extra waits onto NoOp carrier instructions inserted
    just before the offending instruction on the same engine (semantically
    identical: the engine blocks at the same program point)."""
    n_nop = 0
    for fn in nc.m.functions:
        for blk in fn.blocks:
            il = blk.instructions
            idx = 0
            while idx < len(il):
                inst = il[idx]
                si = inst.sync_info
                if si is not None and len(si.on_wait) > limit:
                    waits = list(si.on_wait)
                    extra, keep = waits[:-limit], waits[-limit:]
                    inst.sync_info = mybir.SyncInfo(
                        on_wait=keep, on_update=list(si.on_update)
                    )
                    for w in extra:
                        nop = mybir.InstNoOp(name=f"waitnop-{n_nop}", ins=[],
                                             outs=[])
                        n_nop += 1
                        nop.engine = inst.engine
                        nop.sync_info = mybir.SyncInfo(on_wait=[w], on_update=[])
                        il.insert(idx, nop)
                        idx += 1
                idx += 1
    return n_nop


def _get_nc(split_waits=True):
    key = ("nc", split_waits)
    if key not in _cache:
        nc = bass.Bass("TRN2", debug=False, target_bir_lowering=False,
                       num_devices=NCORES)
        with tile.TileContext(nc) as tc:
            _build_kernel_body(tc)
        if split_waits:
            _split_multi_waits(nc)
        _cache[key] = nc
    return _cache[key]


def _prep_inputs(x, W_qk):
    x = np.asarray(x, dtype=np.float32)
    W = np.asarray(W_qk, dtype=np.float32)
    n = x.shape[0]
    xh = x.reshape(n, H, D)
    nrm = np.sqrt(np.sum(xh * xh, axis=-1, keepdims=True, dtype=np.float32))
    xh = (xh / nrm).astype(np.float32)
    A = np.ascontiguousarray(xh.reshape(n, DIM))
    AT = np.ascontiguousarray(A.T)  # (768, 1024)

    c0i = (C0 * np.eye(64, dtype=np.float32)).astype(np.float32)

    in_maps = []
    for c in range(NCORES):
        wqk = np.zeros((SLOTS, DIM, 128), dtype=np.float32)
        whh = np.zeros((SLOTS, 64, 128), dtype=np.float32)
        ats = np.zeros((SLOTS, 64, N), dtype=np.float32)
        for s in range(SLOTS):
            h = HEAD_MAP[c][s]
            if h < 0:
                h = 0  # dummy slot computes head 0; output ignored
            Wq_h = W[h * D : (h + 1) * D, :]          # (64, 768)
            Wk_h = W[DIM + h * D : DIM + (h + 1) * D, :]
            wqk[s, :, 0:64] = Wq_h.T
            wqk[s, :, 64:128] = Wk_h.T
            whh[s, :, 0:64] = Wq_h[:, h * D : (h + 1) * D]
            whh[s, :, 64:128] = Wk_h[:, h * D : (h + 1) * D]
            ats[s] = AT[h * D : (h + 1) * D, :]
        in_maps.append({
            "at": AT,
            "wqk": np.ascontiguousarray(wqk),
            "whh": np.ascontiguousarray(whh),
            "ats": np.ascontiguousarray(ats),
            "c0i": c0i,
        })
    return in_maps


def kernel(x, mask, W_qk, trace=False):
    nc = _get_nc()
    in_maps = _prep_inputs(x, W_qk)
    res = bass_utils.run_bass_kernel_spmd(
        nc, in_maps, core_ids=list(range(NCORES)), trace=trace
    )
    _cache["last_results"] = res

    out = np.empty((N, DIM), dtype=np.float32)
    for c in range(NCORES):
        for s in range(SLOTS):
            h = HEAD_MAP[c][s]
            if h >= 0:
                out[:, h * D : (h + 1) * D] = res.results[c]["out"][s]
    return out
